# revision 12
# baseline (speedup 1.0000x reference)
"""Trainium2 Bass kernel for nn_BackboneGNN (2x MySAGEConv + BN + readout).

Runs SPMD on 8 NeuronCores. Host side does LAYOUT ONLY: shard edges by
dst-owner core, sort by (src-block group for the int16 gather range [layer1],
dst 128-row window), and pad each (group, window) cell to whole 128-edge
blocks on a schedule shared by all cores (SPMD: one program).

Device side per conv layer (edge phase):
  load x_src (pregathered bf16, layer0) or dma_gather from the AllGather'd
  h table (layer1); add xe (bf16) + relu -> messages; per 128-edge block
  build a one-hot S = (iota == r) on DVE and accumulate
  psum[window 128 rows, D] += S.T @ msg on the TensorEngine; flush windows
  into an SBUF aggregate. No scatter: dma_scatter_add RMWs race on
  duplicate rows (verified on silicon), matmul aggregation does not.

Node phase: agg*inv_cnt -> transpose -> h = aggT.T@WlT + xT.T@WrT + b;
BN stats via ones-matmuls + AllReduce; BN+relu; AllGather h. Layer 1 adds
graph mean-pool via onehot matmul + AllReduce and the readout projection.
"""

import math

import numpy as np
import ml_dtypes

import concourse.bass as bass
import concourse.bacc as bacc
import concourse.mybir as mybir
import concourse.tile as tile

BF16 = ml_dtypes.bfloat16
EPS = 1e-5
SENTINEL = 300.0  # r value for pad slots: matches no iota column


def make_config(N=100000, E=1600000, D=128, C=64, G=128, NC=8, CH=16, RB=32768):
    NL = N // NC
    assert NL * NC == N
    W = (NL + 127) // 128
    NLP = W * 128
    NGRP = (NC * NLP + RB - 1) // RB
    TE = CH * 128
    return dict(N=N, E=E, D=D, C=C, G=G, NC=NC, CH=CH, RB=RB,
                NL=NL, W=W, NLP=NLP, NGRP=NGRP, TE=TE)


# ---------------------------------------------------------------- host layout

def _cell_layout(cnt, CH, ngrp, W):
    """cnt: [NC, ngrp*W] edge counts. Returns (NB per cell [ngrp, W], physical
    block base per cell [ngrp, W], padded total blocks). Each group's block
    span is padded to a CH multiple so edge tiles are group-pure."""
    NB = np.maximum(np.ceil(cnt.max(axis=0) / 128).astype(np.int64), 1)
    NB = NB.reshape(ngrp, W)
    base = np.zeros((ngrp, W), np.int64)
    off = 0
    for g in range(ngrp):
        for w in range(W):
            base[g, w] = off
            off += NB[g, w]
        off = int(math.ceil(off / CH) * CH)
    return NB, base, off


def preprocess(x, xe, edge_index, batch, cfg):
    N, E, D, G, NC = cfg["N"], cfg["E"], cfg["D"], cfg["G"], cfg["NC"]
    NL, W, NLP, NGRP, RB = cfg["NL"], cfg["W"], cfg["NLP"], cfg["NGRP"], cfg["RB"]
    CH, TE = cfg["CH"], cfg["TE"]

    src = np.asarray(edge_index[0], np.int64)
    dst = np.asarray(edge_index[1], np.int64)
    batch = np.asarray(batch, np.int64)

    deg = np.bincount(dst, minlength=N)
    inv_deg = (1.0 / np.maximum(deg, 1)).astype(np.float32)

    core = dst // NL
    dstl = dst - core * NL
    win = dstl // 128
    row = dstl - win * 128  # row within window

    psrc = (src // NL) * NLP + (src % NL)   # row in padded AllGather table
    grp = psrc // RB

    xe_bf = np.asarray(xe, np.float32).astype(BF16)
    x_bf = np.asarray(x, np.float32).astype(BF16)

    def build_layer(use_groups):
        ng = NGRP if use_groups else 1
        gg = grp if use_groups else 0
        cell = (core * ng + gg) * W + win
        ncell = ng * W
        cnt = np.bincount(cell, minlength=NC * ncell).reshape(NC, ncell)
        NB, cbase, nb_totP = _cell_layout(cnt, CH, ng, W)
        NT = nb_totP // CH

        order = np.argsort(cell, kind="stable")
        scell = cell[order]
        starts = np.concatenate(
            [[0], np.cumsum(cnt.reshape(-1))])[:-1].reshape(NC, ncell)
        rank = np.arange(E, dtype=np.int64) - starts[core[order], scell % ncell]
        lc = scell % ncell
        blk = cbase.reshape(-1)[lc] + rank // 128
        part = rank % 128
        return dict(order=order, blk=blk, part=part, NB=NB, cbase=cbase,
                    nb_totP=nb_totP, NT=NT)

    L0 = build_layer(False)
    L1 = build_layer(True)

    per_core = []
    for c in range(NC):
        pc = {}
        # ---- layer 0 arrays
        m = core[L0["order"]] == c
        o = L0["order"][m]
        blk, part = L0["blk"][m], L0["part"][m]
        NT0, NB0P = L0["NT"], L0["nb_totP"]
        xs0 = np.zeros((NT0, 128, CH, D), BF16)
        xe0 = np.zeros((NT0, 128, CH, D), BF16)
        t_, b_ = blk // CH, blk % CH
        xs0[t_, part, b_] = x_bf[src[o]]
        xe0[t_, part, b_] = xe_bf[o]
        r0 = np.full((128, NB0P), SENTINEL, np.float32)
        r0[part, blk] = row[o]
        pc["xs0"] = xs0.reshape(NT0, 128, CH * D)
        pc["xe0"] = xe0.reshape(NT0, 128, CH * D)
        pc["r0"] = r0
        # ---- layer 1 arrays
        m = core[L1["order"]] == c
        o = L1["order"][m]
        blk, part = L1["blk"][m], L1["part"][m]
        NT1, NB1P = L1["NT"], L1["nb_totP"]
        xe1 = np.zeros((NT1, 128, CH, D), BF16)
        t_, b_ = blk // CH, blk % CH
        xe1[t_, part, b_] = xe_bf[o]
        r1 = np.full((128, NB1P), SENTINEL, np.float32)
        r1[part, blk] = row[o]
        gidx_flat = np.zeros((NT1 * TE,), np.int16)
        slot = (blk // CH) * TE + b_ * 128 + part
        gidx_flat[slot] = (psrc[o] - grp[o] * RB).astype(np.int16)
        gw = gidx_flat.reshape(NT1, CH * 8, 16).transpose(0, 2, 1)
        pc["xe1"] = xe1.reshape(NT1, 128, CH * D)
        pc["r1"] = r1
        pc["gidx"] = np.ascontiguousarray(np.tile(gw, (1, 8, 1)))
        per_core.append(pc)

    # node-phase per-core arrays
    x_f32 = np.asarray(x, np.float32)
    for c in range(NC):
        xl = np.zeros((NLP, D), np.float32)
        xl[:NL] = x_f32[c * NL:(c + 1) * NL]
        per_core[c]["x_lT"] = np.ascontiguousarray(
            xl.reshape(W, 128, D).transpose(0, 2, 1))
        ic = np.ones(NLP, np.float32)
        ic[:NL] = inv_deg[c * NL:(c + 1) * NL]
        per_core[c]["inv_cnt"] = np.ascontiguousarray(ic.reshape(W, 128).T)
        oh = np.zeros((NLP, G), np.float32)
        oh[np.arange(NL), batch[c * NL:(c + 1) * NL]] = 1.0
        per_core[c]["onehot"] = oh

    cnt_g = np.bincount(batch, minlength=G).astype(np.float32)
    inv_cnt_g = (1.0 / np.maximum(cnt_g, 1.0)).astype(np.float32)[None, :]
    pad0 = NL - 128 * (W - 1)
    h0_mask = (np.arange(128) < pad0).astype(np.float32)[:, None]
    iota = np.tile(np.arange(128, dtype=np.float32)[None, :], (128, CH))
    for c in range(NC):
        per_core[c]["inv_cnt_g"] = inv_cnt_g
        per_core[c]["h0_mask"] = h0_mask
        per_core[c]["iota"] = iota

    meta = dict(
        NB0=tuple(int(v) for v in L0["NB"][0]),
        NB1=tuple(tuple(int(v) for v in L1["NB"][g]) for g in range(NGRP)),
        NT0=L0["NT"], NT1=L1["NT"],
        NB0P=L0["nb_totP"], NB1P=L1["nb_totP"],
    )
    return per_core, meta


def make_weight_inputs(cfg, Wl0, bl0, Wr0, Wl1, bl1, Wr1, gamma0, beta0, Wp, bp):
    D, C = cfg["D"], cfg["C"]
    return dict(
        Wl0T=np.ascontiguousarray(np.asarray(Wl0, np.float32).T),
        Wr0T=np.ascontiguousarray(np.asarray(Wr0, np.float32).T),
        Wl1T=np.ascontiguousarray(np.asarray(Wl1, np.float32).T),
        Wr1T=np.ascontiguousarray(np.asarray(Wr1, np.float32).T),
        WpT=np.ascontiguousarray(np.asarray(Wp, np.float32).T),
        bl0b=np.ascontiguousarray(
            np.broadcast_to(np.asarray(bl0, np.float32), (128, D))),
        bl1c=np.ascontiguousarray(np.asarray(bl1, np.float32)[:, None]),
        bpc=np.ascontiguousarray(np.asarray(bp, np.float32)[:, None]),
        gamma=np.ascontiguousarray(np.asarray(gamma0, np.float32)[None, :]),
        beta=np.ascontiguousarray(np.asarray(beta0, np.float32)[None, :]),
        ident=np.eye(128, dtype=np.float32),
    )


def _block_schedule(NB, CH):
    """NB: [ngrp][W] blocks per cell. Physical-block schedule with per-group
    CH padding: entries (window, start, stop, first_pass) or None (pad)."""
    sched = []
    ngrp = len(NB)
    for g in range(ngrp):
        for w in range(len(NB[g])):
            nb = NB[g][w]
            for i in range(nb):
                sched.append((w, i == 0, i == nb - 1, g == 0))
        while len(sched) % CH:
            sched.append(None)
    return sched


# ---------------------------------------------------------------- bass program

def build_program(cfg, meta):
    N, D, C, G, NC = cfg["N"], cfg["D"], cfg["C"], cfg["G"], cfg["NC"]
    NL, W, NLP, NGRP, RB = cfg["NL"], cfg["W"], cfg["NLP"], cfg["NGRP"], cfg["RB"]
    CH, TE = cfg["CH"], cfg["TE"]
    NT0, NT1 = meta["NT0"], meta["NT1"]
    NB0P, NB1P = meta["NB0P"], meta["NB1P"]
    f32, bf16, i16 = mybir.dt.float32, mybir.dt.bfloat16, mybir.dt.int16
    AO = mybir.AluOpType
    AF = mybir.ActivationFunctionType
    RG = [list(range(NC))]

    sched0 = _block_schedule([meta["NB0"]], CH)
    sched1 = _block_schedule(meta["NB1"], CH)
    grp_of_tile1 = []
    for g in range(NGRP):
        nt = int(math.ceil(sum(meta["NB1"][g]) / CH))
        grp_of_tile1 += [g] * nt

    nc = bacc.Bacc("TRN2", target_bir_lowering=False, debug=False, num_devices=NC)

    def din(name, shape, dt=f32):
        return nc.dram_tensor(name, list(shape), dt, kind="ExternalInput").ap()

    xs0_d = din("xs0", [NT0, 128, CH * D], bf16)
    xe0_d = din("xe0", [NT0, 128, CH * D], bf16)
    r0_d = din("r0", [128, NB0P])
    xe1_d = din("xe1", [NT1, 128, CH * D], bf16)
    r1_d = din("r1", [128, NB1P])
    gidx_d = din("gidx", [NT1, 128, CH * 8], i16)
    x_lT = din("x_lT", [W, D, 128])
    inv_cnt_d = din("inv_cnt", [128, W])
    onehot_d = din("onehot", [NLP, G])
    inv_cnt_g_d = din("inv_cnt_g", [1, G])
    h0_mask_d = din("h0_mask", [128, 1])
    iota_d = din("iota", [128, CH * 128])
    Wl0T_d = din("Wl0T", [D, D]); Wr0T_d = din("Wr0T", [D, D])
    Wl1T_d = din("Wl1T", [D, D]); Wr1T_d = din("Wr1T", [D, D])
    WpT_d = din("WpT", [D, C])
    bl0b_d = din("bl0b", [128, D]); bl1c_d = din("bl1c", [D, 1])
    bpc_d = din("bpc", [C, 1])
    gamma_d = din("gamma", [1, D]); beta_d = din("beta", [1, D])
    ident_d = din("ident", [128, 128])

    houtT = nc.dram_tensor("houtT", [C, NLP], f32, kind="ExternalOutput").ap()
    gT = nc.dram_tensor("gT", [D, G], f32, kind="ExternalOutput").ap()

    debug = cfg.get("debug", False)
    if debug:
        d_agg0 = nc.dram_tensor("d_agg0", [128, W * D], f32,
                                kind="ExternalOutput").ap()
        d_stats = nc.dram_tensor("d_stats", [2, D], f32, kind="ExternalOutput").ap()
        d_hbnT = nc.dram_tensor("d_hbnT", [128, NLP], f32,
                                kind="ExternalOutput").ap()
        d_hfull = nc.dram_tensor("d_hfull", [NC * NLP, D], f32,
                                 kind="ExternalOutput").ap()
        d_agg1 = nc.dram_tensor("d_agg1", [128, W * D], f32,
                                kind="ExternalOutput").ap()

    # internal DRAM
    h0_dram = nc.dram_tensor("h0_dram", [128, NLP], f32).ap()
    stats_in = nc.dram_tensor("stats_in", [2, D], f32).ap()
    stats_out = nc.dram_tensor("stats_out", [2, D], f32).ap()
    ag_in = nc.dram_tensor("ag_in", [NLP, D], bf16).ap()
    h_full = nc.dram_tensor("h_full", [NC * NLP, D], bf16).ap()
    pool_in = nc.dram_tensor("pool_in", [D, G], f32).ap()
    pool_out = nc.dram_tensor("pool_out", [D, G], f32).ap()

    tbl_rows = [min(RB, NC * NLP - g * RB) for g in range(NGRP)]

    with tile.TileContext(nc) as tc:
        with (
            tc.tile_pool(name="cpool", bufs=1) as cpool,
            tc.tile_pool(name="resid", bufs=1) as resid,
        ):
            Wl0T = cpool.tile([D, D], f32); nc.sync.dma_start(Wl0T[:], Wl0T_d[:])
            Wr0T = cpool.tile([D, D], f32); nc.sync.dma_start(Wr0T[:], Wr0T_d[:])
            Wl1T = cpool.tile([D, D], f32); nc.sync.dma_start(Wl1T[:], Wl1T_d[:])
            Wr1T = cpool.tile([D, D], f32); nc.sync.dma_start(Wr1T[:], Wr1T_d[:])
            WpT = cpool.tile([D, C], f32); nc.sync.dma_start(WpT[:], WpT_d[:])
            bl0b = cpool.tile([128, D], f32); nc.sync.dma_start(bl0b[:], bl0b_d[:])
            bl1c = cpool.tile([D, 1], f32); nc.sync.dma_start(bl1c[:], bl1c_d[:])
            bpc = cpool.tile([C, 1], f32); nc.sync.dma_start(bpc[:], bpc_d[:])
            gamma_t = cpool.tile([1, D], f32); nc.sync.dma_start(gamma_t[:], gamma_d[:])
            beta_t = cpool.tile([1, D], f32); nc.sync.dma_start(beta_t[:], beta_d[:])
            ident = cpool.tile([128, 128], f32); nc.sync.dma_start(ident[:], ident_d[:])
            icnt = cpool.tile([128, W], f32); nc.sync.dma_start(icnt[:], inv_cnt_d[:])
            icg = cpool.tile([1, G], f32); nc.sync.dma_start(icg[:], inv_cnt_g_d[:])
            mask0 = cpool.tile([128, 1], f32); nc.sync.dma_start(mask0[:], h0_mask_d[:])
            iota_t = cpool.tile([128, CH * 128], f32)
            nc.sync.dma_start(iota_t[:], iota_d[:])
            r0_t = cpool.tile([128, NB0P], f32); nc.sync.dma_start(r0_t[:], r0_d[:])
            r1_t = cpool.tile([128, NB1P], f32); nc.sync.dma_start(r1_t[:], r1_d[:])
            ones_c = cpool.tile([128, 1], f32); nc.gpsimd.memset(ones_c[:], 1.0)
            ones_r = cpool.tile([1, 128], f32); nc.gpsimd.memset(ones_r[:], 1.0)

            hbnT_res = resid.tile([128, NLP], f32)
            agg_res = resid.tile([128, W, D], f32)  # window-major aggregate

            def edge_phase(NT, sched, load_src, xe_dram, r_t, ep, ps, tagp):
                win_ps = {}
                for t in range(NT):
                    xsrc = load_src(t, ep)
                    xet = ep.tile([128, CH * D], bf16, name=f"{tagp}xe{t}",
                                  tag="xe")
                    nc.sync.dma_start(xet[:], xe_dram[t])
                    msg = ep.tile([128, CH * D], bf16, name=f"{tagp}msg{t}",
                                  tag="msg")
                    nc.vector.tensor_tensor(msg[:], xsrc[:], xet[:], AO.add)
                    nc.scalar.activation(msg[:], msg[:], AF.Relu)
                    S = ep.tile([128, CH, 128], bf16, name=f"{tagp}S{t}", tag="S")
                    Sf = S.rearrange("p a b -> p (a b)")
                    rsl = r_t[:, t * CH:(t + 1) * CH]
                    rbc = bass.AP(
                        tensor=rsl.tensor, offset=rsl.offset,
                        ap=[list(rsl.ap[0]), [rsl.ap[1][0], CH], [0, 128]])
                    nc.vector.tensor_tensor(Sf[:], iota_t[:], rbc, AO.is_equal)
                    m3 = msg.rearrange("p (a b) -> p a b", a=CH)
                    for b in range(CH):
                        B = t * CH + b
                        if B >= len(sched) or sched[B] is None:
                            continue
                        wdw, first, last, gfirst = sched[B]
                        key = wdw % 2
                        if first:
                            win_ps[key] = ps.tile(
                                [128, D], f32, name=f"{tagp}w{B}", tag=f"win{key}")
                        wp = win_ps[key]
                        nc.tensor.matmul(wp[:], S[:, b, :], m3[:, b, :],
                                         start=first, stop=last)
                        if last:
                            slab = agg_res[:, wdw, :]
                            if gfirst:
                                nc.vector.tensor_copy(slab, wp[:])
                            else:
                                nc.vector.tensor_tensor(slab, slab, wp[:], AO.add)

            # ---------------- layer 0 edge phase ----------------
            with (
                tc.tile_pool(name="edge0", bufs=3) as ep,
                tc.tile_pool(name="psw0", bufs=2, space="PSUM") as ps,
            ):
                def load0(t, ep):
                    xs = ep.tile([128, CH * D], bf16, name=f"xs{t}", tag="xs")
                    nc.sync.dma_start(xs[:], xs0_d[t])
                    return xs
                edge_phase(NT0, sched0, load0, xe0_d, r0_t, ep, ps, "a")

            # ---------------- layer 0 node phase ----------------
            with (
                tc.tile_pool(name="node0", bufs=3) as np0,
                tc.tile_pool(name="psum0", bufs=2, space="PSUM") as ps0,
            ):
                st1 = np0.tile([1, D], f32, bufs=1)
                st2 = np0.tile([1, D], f32, bufs=1)
                for j in range(W):
                    r0c, r1c = j * 128, (j + 1) * 128
                    am = np0.tile([128, D], f32, name=f"am{j}", tag="am")
                    nc.vector.tensor_scalar_mul(am[:], agg_res[:, j, :],
                                                icnt[:, j:j + 1])
                    amT_p = ps0.tile([D, 128], f32, name=f"amTp{j}", tag="amTp")
                    nc.tensor.transpose(amT_p[:], am[:], ident[:])
                    amT = np0.tile([D, 128], f32, name=f"amT{j}", tag="amT")
                    nc.vector.tensor_copy(amT[:], amT_p[:])

                    xlt = np0.tile([D, 128], f32, name=f"xlt{j}", tag="xlt")
                    nc.sync.dma_start(xlt[:], x_lT[j])

                    h0_p = ps0.tile([128, D], f32, name=f"h0p{j}", tag="h0p")
                    nc.tensor.matmul(h0_p[:], amT[:], Wl0T[:], start=True, stop=False)
                    nc.tensor.matmul(h0_p[:], xlt[:], Wr0T[:], start=False, stop=True)

                    h0_t = np0.tile([128, D], f32, name=f"h0_{j}", tag="h0")
                    nc.vector.tensor_tensor(h0_t[:], h0_p[:], bl0b[:], AO.add)
                    if j == W - 1:
                        nc.vector.tensor_scalar_mul(h0_t[:], h0_t[:], mask0[:, 0:1])
                    nc.sync.dma_start(h0_dram[:, r0c:r1c], h0_t[:])

                    sq = np0.tile([128, D], f32, name=f"sq{j}", tag="sq")
                    nc.vector.tensor_mul(sq[:], h0_t[:], h0_t[:])
                    s1p = ps0.tile([1, D], f32, name=f"s1p{j}", tag="s1p")
                    nc.tensor.matmul(s1p[:], ones_c[:], h0_t[:], start=True,
                                     stop=True)
                    s2p = ps0.tile([1, D], f32, name=f"s2p{j}", tag="s2p")
                    nc.tensor.matmul(s2p[:], ones_c[:], sq[:], start=True,
                                     stop=True)
                    if j == 0:
                        nc.vector.tensor_copy(st1[:], s1p[:])
                        nc.vector.tensor_copy(st2[:], s2p[:])
                    else:
                        nc.vector.tensor_tensor(st1[:], st1[:], s1p[:], AO.add)
                        nc.vector.tensor_tensor(st2[:], st2[:], s2p[:], AO.add)

                nc.sync.dma_start(stats_in[0:1, :], st1[:])
                nc.sync.dma_start(stats_in[1:2, :], st2[:])

            nc.gpsimd.collective_compute(
                "AllReduce", AO.add, replica_groups=RG,
                ins=[stats_in.opt()], outs=[stats_out.opt()])

            # ---------------- BN finalize + apply + transpose ----------------
            with (
                tc.tile_pool(name="bnp", bufs=3) as bp_,
                tc.tile_pool(name="psumb", bufs=2, space="PSUM") as psb,
            ):
                sg1 = bp_.tile([1, D], f32)
                nc.sync.dma_start(sg1[:], stats_out[0:1, :])
                sg2 = bp_.tile([1, D], f32)
                nc.sync.dma_start(sg2[:], stats_out[1:2, :])
                mean = bp_.tile([1, D], f32)
                nc.scalar.mul(mean[:], sg1[:], 1.0 / N)
                ex2 = bp_.tile([1, D], f32)
                nc.scalar.mul(ex2[:], sg2[:], 1.0 / N)
                var = bp_.tile([1, D], f32)
                nc.vector.tensor_mul(var[:], mean[:], mean[:])
                nc.vector.tensor_sub(var[:], ex2[:], var[:])
                nc.vector.tensor_scalar_add(var[:], var[:], EPS)
                std = bp_.tile([1, D], f32)
                nc.scalar.activation(std[:], var[:], AF.Sqrt)
                rstd = bp_.tile([1, D], f32)
                nc.vector.reciprocal(rstd[:], std[:])
                scl = bp_.tile([1, D], f32)
                nc.vector.tensor_mul(scl[:], gamma_t[:], rstd[:])
                shf = bp_.tile([1, D], f32)
                nc.vector.tensor_mul(shf[:], mean[:], scl[:])
                nc.vector.tensor_sub(shf[:], beta_t[:], shf[:])

                scb_p = psb.tile([128, D], f32, bufs=1)
                nc.tensor.matmul(scb_p[:], ones_r[:], scl[:], start=True, stop=True)
                shb_p = psb.tile([128, D], f32, bufs=1)
                nc.tensor.matmul(shb_p[:], ones_r[:], shf[:], start=True, stop=True)
                scb = bp_.tile([128, D], f32, bufs=1)
                nc.vector.tensor_copy(scb[:], scb_p[:])
                shb = bp_.tile([128, D], f32, bufs=1)
                nc.vector.tensor_copy(shb[:], shb_p[:])

                for j in range(W):
                    r0c, r1c = j * 128, (j + 1) * 128
                    h0l = bp_.tile([128, D], f32, name=f"h0l{j}", tag="h0l")
                    nc.sync.dma_start(h0l[:], h0_dram[:, r0c:r1c])
                    hbn = bp_.tile([128, D], f32, name=f"hbn{j}", tag="hbn")
                    nc.vector.tensor_tensor(hbn[:], h0l[:], scb[:], AO.mult)
                    nc.vector.tensor_tensor(hbn[:], hbn[:], shb[:], AO.add)
                    nc.vector.tensor_scalar_max(hbn[:], hbn[:], 0.0)
                    nc.gpsimd.dma_start(ag_in[r0c:r1c, :], hbn[:])  # f32->bf16
                    hbnT_p = psb.tile([D, 128], f32, name=f"hbnTp{j}", tag="hbnTp")
                    nc.tensor.transpose(hbnT_p[:], hbn[:], ident[:])
                    nc.vector.tensor_copy(hbnT_res[:, r0c:r1c], hbnT_p[:])

            nc.gpsimd.collective_compute(
                "AllGather", AO.bypass, replica_groups=RG,
                ins=[ag_in.opt()], outs=[h_full.opt()])

            if debug:
                nc.sync.dma_start(
                    d_agg0[:], agg_res.rearrange("p a b -> p (a b)")[:])
                nc.sync.dma_start(d_stats[:], stats_out[:])
                nc.sync.dma_start(d_hbnT[:], hbnT_res[:])
                dhf = bp_ if False else None
                with tc.tile_pool(name="dbg", bufs=2) as dp:
                    nchunk = (NC * NLP) // 128
                    for k in range(nchunk):
                        tte = dp.tile([128, D], bf16, name=f"dh{k}", tag="dh")
                        nc.sync.dma_start(tte[:], h_full[k * 128:(k + 1) * 128, :])
                        ttf = dp.tile([128, D], f32, name=f"dhf{k}", tag="dhf")
                        nc.vector.tensor_copy(ttf[:], tte[:])
                        nc.sync.dma_start(d_hfull[k * 128:(k + 1) * 128, :], ttf[:])

            # ---------------- layer 1 edge phase ----------------
            with (
                tc.tile_pool(name="edge1", bufs=3) as ep,
                tc.tile_pool(name="psw1", bufs=2, space="PSUM") as ps,
            ):
                GMAX = 1024  # dma_gather with more idxs crashes NRT (HW-tested)

                def load1(t, ep):
                    gi = ep.tile([128, CH * 8], i16, name=f"gi{t}", tag="gi",
                                 bufs=2)
                    nc.sync.dma_start(gi[:], gidx_d[t])
                    g = grp_of_tile1[t]
                    gat = ep.tile([128, CH, D], bf16, name=f"gat{t}", tag="gat")
                    tbl = h_full[g * RB:g * RB + tbl_rows[g], :]
                    nsub = max(TE // GMAX, 1)
                    nidx = TE // nsub
                    cw = nidx // 16          # idx columns per sub-gather
                    cb = nidx // 128         # out blocks per sub-gather
                    for u in range(nsub):
                        nc.gpsimd.dma_gather(
                            out_ap=gat[:, u * cb:(u + 1) * cb, :],
                            in_ap=tbl,
                            idxs_ap=gi[:, u * cw:(u + 1) * cw],
                            num_idxs=nidx, num_idxs_reg=nidx,
                            elem_size=D)
                    return gat.rearrange("p a b -> p (a b)")
                edge_phase(NT1, sched1, load1, xe1_d, r1_t, ep, ps, "b")

            # ---------------- layer 1 node phase + pool + readout ------------
            with (
                tc.tile_pool(name="node1", bufs=3) as np1,
                tc.tile_pool(name="psum1", bufs=2, space="PSUM") as ps1,
            ):
                poolacc = np1.tile([D, G], f32, bufs=1)
                for j in range(W):
                    r0c, r1c = j * 128, (j + 1) * 128
                    am = np1.tile([128, D], f32, name=f"bm{j}", tag="am")
                    nc.vector.tensor_scalar_mul(am[:], agg_res[:, j, :],
                                                icnt[:, j:j + 1])
                    amT_p = ps1.tile([D, 128], f32, name=f"bmTp{j}", tag="amTp")
                    nc.tensor.transpose(amT_p[:], am[:], ident[:])
                    amT = np1.tile([D, 128], f32, name=f"bmT{j}", tag="amT")
                    nc.vector.tensor_copy(amT[:], amT_p[:])

                    h1_p = ps1.tile([D, 128], f32, name=f"h1p{j}", tag="h1p")
                    nc.tensor.matmul(h1_p[:], Wl1T[:], amT[:], start=True,
                                     stop=False)
                    nc.tensor.matmul(h1_p[:], Wr1T[:], hbnT_res[:, r0c:r1c],
                                     start=False, stop=True)
                    h1 = np1.tile([D, 128], f32, name=f"h1_{j}", tag="h1")
                    nc.vector.tensor_scalar_add(h1[:], h1_p[:], bl1c[:, 0:1])

                    h1n_p = ps1.tile([128, D], f32, name=f"h1np{j}", tag="h1np",
                                     bufs=1)
                    nc.tensor.transpose(h1n_p[:], h1[:], ident[:])
                    h1n = np1.tile([128, D], f32, name=f"h1n{j}", tag="h1n")
                    nc.vector.tensor_copy(h1n[:], h1n_p[:])
                    oh = np1.tile([128, G], f32, name=f"oh{j}", tag="oh")
                    nc.sync.dma_start(oh[:], onehot_d[r0c:r1c, :])
                    pp = ps1.tile([D, G], f32, name=f"pp{j}", tag="pp", bufs=1)
                    nc.tensor.matmul(pp[:], h1n[:], oh[:], start=True, stop=True)
                    if j == 0:
                        nc.vector.tensor_copy(poolacc[:], pp[:])
                    else:
                        nc.vector.tensor_tensor(poolacc[:], poolacc[:], pp[:],
                                                AO.add)

                    nc.vector.tensor_scalar_max(h1[:], h1[:], 0.0)
                    ho_p = ps1.tile([C, 128], f32, name=f"hop{j}", tag="hop",
                                    bufs=1)
                    nc.tensor.matmul(ho_p[:], WpT[:], h1[:], start=True, stop=True)
                    ho = np1.tile([C, 128], f32, name=f"ho{j}", tag="ho")
                    nc.vector.tensor_scalar_add(ho[:], ho_p[:], bpc[:, 0:1])
                    nc.sync.dma_start(houtT[:, r0c:r1c], ho[:])

                nc.sync.dma_start(pool_in[:], poolacc[:])

            if debug:
                nc.sync.dma_start(
                    d_agg1[:], agg_res.rearrange("p a b -> p (a b)")[:])

            nc.gpsimd.collective_compute(
                "AllReduce", AO.add, replica_groups=RG,
                ins=[pool_in.opt()], outs=[pool_out.opt()])

            with (
                tc.tile_pool(name="fin", bufs=1) as fp,
                tc.tile_pool(name="psumf", bufs=1, space="PSUM") as psf,
            ):
                po = fp.tile([D, G], f32)
                nc.sync.dma_start(po[:], pool_out[:])
                icgb_p = psf.tile([128, G], f32)
                nc.tensor.matmul(icgb_p[:], ones_r[:], icg[:], start=True,
                                 stop=True)
                gt = fp.tile([D, G], f32)
                nc.vector.tensor_tensor(gt[:], po[:], icgb_p[:D, :], AO.mult)
                nc.sync.dma_start(gT[:], gt[:])

    nc.compile()
    return nc


# ---------------------------------------------------------------- entry point

_CACHE = {}


def _get_program(cfg, meta):
    key = (tuple(sorted((k, v) for k, v in cfg.items())),
           meta["NB0"], meta["NB1"])
    if key not in _CACHE:
        _CACHE[key] = build_program(cfg, meta)
    return _CACHE[key]


def run(inputs, cfg, run_fn=None):
    per_core, meta = preprocess(
        inputs["x"], inputs["xe"], inputs["edge_index"], inputs["batch"], cfg)
    wts = make_weight_inputs(
        cfg, inputs["Wl0"], inputs["bl0"], inputs["Wr0"], inputs["Wl1"],
        inputs["bl1"], inputs["Wr1"], inputs["gamma0"], inputs["beta0"],
        inputs["Wp"], inputs["bp"])
    in_maps = [{**pc, **wts} for pc in per_core]

    nc = _get_program(cfg, meta)

    if run_fn is None:
        from concourse.bass_utils import run_bass_kernel_spmd
        res = run_bass_kernel_spmd(nc, in_maps, list(range(cfg["NC"]))).results
    else:
        res = run_fn(nc, in_maps)

    NL, NC = cfg["NL"], cfg["NC"]
    h = np.concatenate([res[c]["houtT"][:, :NL].T for c in range(NC)], axis=0)
    g = res[0]["gT"].T[:cfg["G"], :]
    return np.ascontiguousarray(h), np.ascontiguousarray(g)


def kernel(**inputs):
    cfg = make_config()
    return run(inputs, cfg)
